# revision 1
# baseline (speedup 1.0000x reference)
"""Trainium2 Bass kernel for nn_DEQLatentSpaceOpt (DDIM trajectory DEQ iteration).

Computation (reference):
    xT = x[0:1]
    repeat 3x:  et = conv3x3(xt[:T]) + temb[t][:,:,None,None]
                xt_next = alpha_ratio*xT + epc * cumsum(et_coeff * et, axis=0)
                xt = concat([xT, xt_next])

Sharding: T=1000 trajectory rows split as 125 rows per core across 8 cores.
Per core, images are packed 3-per-partition-group: partition p = 3g + c
(g in 0..41 group, c channel), image local index l = 3g + j (slot j in 0..2).
The 3x3 conv runs on TensorE as 9 accumulating block-diagonal matmuls over a
row-padded (stride-66) bf16 image layout; shifted taps are plain AP offsets.
The cumsum along the trajectory + all per-timestep coefficients + the
cross-core carry + the alpha_ratio*xT term are folded into host-precomputed
triangular matmul weights (PE does all the math; fp32 PSUM accumulation).
Cross-core prefix: one 8-rank AllGather of per-core totals per iteration.
"""

import numpy as np
import ml_dtypes

import jax
import concourse.bacc as bacc
import concourse.mybir as mybir
import concourse.tile as tile
from concourse.bass_interp import get_hw_module
from concourse import bass2jax

BF16 = mybir.dt.bfloat16
F32 = mybir.dt.float32

N_CORES = 8
T = 1000
C = 3
HW = 4096  # 64*64
TLOC = T // N_CORES  # 125 rows per core
G = 42  # partition groups; partition p = 3g + c, 126 used of 128
S = 3  # image slots per partition (42*3 = 126 slots >= 125 images)
N_ITER = 3

# padded image layout per partition: row stride 66 (1 left pad + 64 px + 1
# right pad), one 66-wide gap row between images, one lead gap row.
ROWS = S * 65 + 1  # 196
RW = 66
TAPS = [(dy, dx) for dy in (-1, 0, 1) for dx in (-1, 0, 1)]
CHUNK_ROWS = 8  # conv matmul chunk: 8 image rows x 64 px = 512 cols
NCH = 64 // CHUNK_ROWS  # 8 chunks per image slot
PS_GRP = 2  # psum tile holds 2 chunks (1024 f32 = 2 banks)

_compiled = None


def _build_module(sim_mode=False):
    """sim_mode: single-core variant with the AllGather replaced by
    byte-equivalent local DMAs, for TimelineSim cost estimation only."""
    nc = bacc.Bacc(
        "TRN2",
        target_bir_lowering=False,
        debug=False,
        num_devices=1 if sim_mode else N_CORES,
    )

    # I/O
    x_arr = nc.dram_tensor("x_arr", [128, S, HW], BF16, kind="ExternalInput").ap()
    xt_bf = nc.dram_tensor("xt_bf", [C, HW], BF16, kind="ExternalInput").ap()
    w9 = nc.dram_tensor("w9", [9, 128, 128], BF16, kind="ExternalInput").ap()
    triw = nc.dram_tensor("triw", [9, 128, 128], BF16, kind="ExternalInput").ap()
    cxw = nc.dram_tensor("cxw", [S, 27, 128], BF16, kind="ExternalInput").ap()
    totw = nc.dram_tensor("totw", [S, 128, C], BF16, kind="ExternalInput").ap()
    biasw = nc.dram_tensor("biasw", [128, S], F32, kind="ExternalInput").ap()
    out_arr = nc.dram_tensor("out_arr", [128, S, HW], F32, kind="ExternalOutput").ap()

    TRI_IDX = {(j, l): 3 * j + l for j in range(S) for l in range(S)}

    with tile.TileContext(nc) as tc:
        with (
            tc.tile_pool(name="persist", bufs=1) as pp,
            tc.tile_pool(name="pconv", bufs=2, space="PSUM") as pconv,
            tc.tile_pool(name="pmisc", bufs=2, space="PSUM") as pmisc,
            tc.tile_pool(name="dram", bufs=2, space="DRAM") as dp,
        ):
            # persistent tiles
            convin = pp.tile([128, ROWS, RW], BF16, tag="convin")
            stag = pp.tile([128, S, HW], F32, tag="stag")
            e = pp.tile([128, S, HW], BF16, tag="e")
            rhs_cx = pp.tile([27, HW], BF16, tag="rhs_cx")
            agin_s = pp.tile([C, HW], BF16, tag="agin_s")
            w9s = pp.tile([128, 9, 128], BF16, tag="w9s")
            tris = pp.tile([128, 9, 128], BF16, tag="tris")
            cxs = pp.tile([27, S, 128], BF16, tag="cxs")
            tots = pp.tile([128, S, C], BF16, tag="tots")
            biass = pp.tile([128, S], F32, tag="biass")

            # zero only the pad regions of the conv input once (pixel areas
            # are fully overwritten by DMA/writeback; pads must stay zero)
            nc.gpsimd.memset(convin[:, :, 0:66:65], 0.0)  # x pads
            for gr in range(0, ROWS, 65):  # lead + inter-image gap rows
                nc.gpsimd.memset(convin[:, gr], 0.0)

            # load coefficients (w9/tris/cxs have leading dim as free axis on
            # 128 partitions; DMA per plane keeps partition dim = 128)
            for i in range(9):
                nc.sync.dma_start(w9s[:, i], w9[i])
            nc.sync.dma_start(biass[:], biasw[:])

            # load x (already bf16, host-quantized) straight into the padded
            # conv-input layout, in slot-quarters so early conv matmuls can
            # start while later pieces still load
            NH = 4
            for j in range(S):
                r0 = 1 + 65 * j
                for h in range(NH):
                    hw2 = HW // NH
                    rh = 64 // NH
                    nc.sync.dma_start(
                        convin[:, r0 + rh * h : r0 + rh * (h + 1), 1:65],
                        x_arr[:, j, h * hw2 : (h + 1) * hw2].rearrange(
                            "p (a b) -> p a b", b=64
                        ),
                    )

            for i in range(9):
                nc.sync.dma_start(tris[:, i], triw[i])
            for j in range(S):
                nc.sync.dma_start(cxs[:, j], cxw[j])
                nc.sync.dma_start(tots[:, j], totw[j])
            nc.sync.dma_start(rhs_cx[24:27, :], xt_bf[:])

            for it in range(N_ITER):
                last = it == N_ITER - 1

                # ---- conv (9 shifted block-diag matmuls per chunk) and
                # totals, interleaved per column-chunk-group so the
                # AllGather input is ready right after the last conv ----
                for cg in range(NCH // PS_GRP):  # chunk groups of 2
                    c0 = cg * PS_GRP * 512
                    for j in range(S):
                        r0 = 1 + 65 * j
                        pt = pconv.tile([128, PS_GRP * 512], F32, tag="pconv")
                        for ci in range(PS_GRP):
                            ch = cg * PS_GRP + ci
                            rr = r0 + ch * CHUNK_ROWS
                            for ti, (dy, dx) in enumerate(TAPS):
                                nc.tensor.matmul(
                                    pt[:, ci * 512 : (ci + 1) * 512],
                                    w9s[:, ti],
                                    convin[
                                        :,
                                        rr + dy : rr + CHUNK_ROWS + dy,
                                        1 + dx : 65 + dx,
                                    ],
                                    start=(ti == 0),
                                    stop=(ti == 8),
                                )
                        # evac: copy + per-partition temb bias -> e (bf16);
                        # alternate ACT/DVE to balance engine load
                        if (j * (NCH // PS_GRP) + cg) % 2 == 0:
                            nc.scalar.activation(
                                e[:, j, c0 : c0 + PS_GRP * 512],
                                pt[:],
                                mybir.ActivationFunctionType.Identity,
                                bias=biass[:, j : j + 1],
                            )
                        else:
                            nc.vector.tensor_scalar(
                                e[:, j, c0 : c0 + PS_GRP * 512],
                                pt[:],
                                biass[:, j : j + 1],
                                None,
                                mybir.AluOpType.add,
                            )
                    # totals for the two column chunks of this group
                    for ci in range(PS_GRP):
                        ch = cg * PS_GRP + ci
                        ptt = pmisc.tile([C, 512], F32, tag="pmisc")
                        for l in range(S):
                            nc.tensor.matmul(
                                ptt[:],
                                tots[:, l],
                                e[:, l, ch * 512 : (ch + 1) * 512],
                                start=(l == 0),
                                stop=(l == S - 1),
                            )
                        nc.vector.tensor_copy(
                            agin_s[:, ch * 512 : (ch + 1) * 512], ptt[:]
                        )
                ag_in = dp.tile([C, HW], BF16, tag="ag_in")
                ag_out = dp.tile([N_CORES * C, HW], BF16, tag="ag_out")
                nc.sync.dma_start(ag_in[:], agin_s[:])
                if sim_mode:
                    for r in range(N_CORES):
                        nc.sync.dma_start(ag_out[3 * r : 3 * r + 3, :], ag_in[:])
                else:
                    nc.gpsimd.collective_compute(
                        "AllGather",
                        mybir.AluOpType.bypass,
                        replica_groups=[list(range(N_CORES))],
                        ins=[ag_in.opt()],
                        outs=[ag_out.opt()],
                    )
                nc.sync.dma_start(rhs_cx[0:24, :], ag_out[:])

                # ---- combine: carry/xT + triangular cumsum matmuls ----
                # j=2 weights are pre-shifted by +3 output partitions and
                # carry the boundary row in columns 0..2 (see _build_inputs),
                # so every writeback is partition-0-aligned.
                for j in range(S):
                    for cg in range(NCH // PS_GRP):
                        pc = pmisc.tile([128, PS_GRP * 512], F32, tag="pmisc")
                        for ci in range(PS_GRP):
                            c0 = (cg * PS_GRP + ci) * 512
                            sl = slice(ci * 512, (ci + 1) * 512)
                            # tri matmuls first (no AllGather dependency —
                            # they overlap the collective), carry/xT last
                            for l in range(S):
                                nc.tensor.matmul(
                                    pc[:, sl],
                                    tris[:, TRI_IDX[(j, l)]],
                                    e[:, l, c0 : c0 + 512],
                                    start=(l == 0),
                                    stop=False,
                                )
                            nc.tensor.matmul(
                                pc[:, sl],
                                cxs[:, j],
                                rhs_cx[:, c0 : c0 + 512],
                                start=False,
                                stop=True,
                            )
                        # writeback
                        rows = PS_GRP * CHUNK_ROWS
                        c0 = cg * PS_GRP * 512
                        use_act = (j * (NCH // PS_GRP) + cg) % 2 == 1
                        if last:
                            if use_act:
                                nc.scalar.activation(
                                    stag[:, j, c0 : c0 + PS_GRP * 512],
                                    pc[:],
                                    mybir.ActivationFunctionType.Copy,
                                )
                            else:
                                nc.vector.tensor_copy(
                                    stag[:, j, c0 : c0 + PS_GRP * 512], pc[:]
                                )
                            # stream this chunk out while later chunks compute
                            nc.sync.dma_start(
                                out_arr[:, j, c0 : c0 + PS_GRP * 512],
                                stag[:, j, c0 : c0 + PS_GRP * 512],
                            )
                        else:
                            # image l=3g+j -> next xt image l+1 (slot j+1, or
                            # slot 0 via the pre-shifted j=2 weights)
                            jd = j + 1 if j < S - 1 else 0
                            rr = 1 + 65 * jd + cg * rows
                            if use_act:
                                nc.scalar.activation(
                                    convin[0:126, rr : rr + rows, 1:65],
                                    pc[0:126].rearrange("p (a b) -> p a b", b=64),
                                    mybir.ActivationFunctionType.Copy,
                                )
                            else:
                                nc.vector.tensor_copy(
                                    convin[0:126, rr : rr + rows, 1:65],
                                    pc[0:126].rearrange("p (a b) -> p a b", b=64),
                                )


    nc.compile()
    nc.m = get_hw_module(nc.m)
    return nc


def _build_inputs(x, alpha_ratio, et_coeff, et_prevsum_coeff, conv_w, temb, t):
    """Host-side coefficient precompute; returns per-core in_maps."""
    ar = np.asarray(alpha_ratio, np.float64).reshape(T)
    etc = np.asarray(et_coeff, np.float64).reshape(T)
    epc = np.asarray(et_prevsum_coeff, np.float64).reshape(T)
    temb = np.asarray(temb, np.float32)
    t = np.asarray(t).astype(np.int64)
    conv_w = np.asarray(conv_w, np.float32)
    x = np.asarray(x, np.float32)
    tembsel = temb[t]  # [T, C] bias per trajectory row

    bf = ml_dtypes.bfloat16

    # shared: conv tap weights, block-diagonal [3g+ci, 3g+co]
    w9 = np.zeros((9, 128, 128), np.float32)
    for ti, (dy, dx) in enumerate(TAPS):
        blk = conv_w[:, :, dy + 1, dx + 1].T  # [ci, co]
        for g in range(G):
            w9[ti, 3 * g : 3 * g + 3, 3 * g : 3 * g + 3] = blk
    w9 = w9.astype(bf)

    xt_b = x[0].reshape(C, HW).astype(bf)

    gs = np.arange(G)
    in_maps = []
    for k in range(N_CORES):
        o = k * TLOC

        def idx(g, j):
            return o + 3 * g + j

        def valid(g, j):
            return 3 * g + j <= TLOC - 1

        vmask = np.array([[valid(g, j) for j in range(S)] for g in range(G)])

        # j=2 combine outputs are shifted +3 partitions (next xt slot (g+1,0))
        # and columns 0..2 hold the boundary row xt_next[o-1].
        def ocol(g, j):
            return 3 * (g + 1) if j == S - 1 else 3 * g

        tri = np.zeros((9, 128, 128), np.float32)
        for j in range(S):
            for l in range(S):
                ti = 3 * j + l
                for g in range(G):
                    if not vmask[g, j]:
                        continue
                    glim = g + 1 if l <= j else g  # 3g'+l <= 3g+j
                    if glim == 0:
                        continue
                    gp = gs[:glim]
                    vv = vmask[gp, l]
                    w = etc[idx(gp, l)] * epc[idx(g, j)] * vv
                    oc = ocol(g, j)
                    if oc + 3 > 128:
                        continue
                    for c in range(C):
                        tri[ti, 3 * gp + c, oc + c] = w

        cx = np.zeros((S, 27, 128), np.float32)
        for j in range(S):
            for g in range(G):
                if not vmask[g, j]:
                    continue
                oc = ocol(g, j)
                if oc + 3 > 128:
                    continue
                for c in range(C):
                    cx[j, 3 * np.arange(k) + c, oc + c] = epc[idx(g, j)]
                    cx[j, 24 + c, oc + c] = ar[idx(g, j)]
        # boundary row -> j=2 columns 0..2
        epc_b = epc[o - 1] if k > 0 else 0.0
        ar_b = ar[o - 1] if k > 0 else 1.0
        for c in range(C):
            cx[S - 1, 3 * np.arange(k) + c, c] = epc_b
            cx[S - 1, 24 + c, c] = ar_b

        tot = np.zeros((S, 128, C), np.float32)
        for l in range(S):
            for g in range(G):
                if vmask[g, l]:
                    for c in range(C):
                        tot[l, 3 * g + c, c] = etc[idx(g, l)]

        bias = np.zeros((128, S), np.float32)
        for j in range(S):
            for g in range(G):
                if vmask[g, j]:
                    bias[3 * g : 3 * g + 3, j] = tembsel[idx(g, j)]

        xa = np.zeros((128, S, HW), bf)
        for j in range(S):
            rows = o + 3 * gs + j  # x row index for slot (g, j); <= 1000
            xa[3 * gs[:, None] + np.arange(C), j] = x[rows].reshape(G, C, HW)

        in_maps.append(
            {
                "x_arr": xa,
                "xt_bf": xt_b,
                "w9": w9,
                "triw": tri.astype(bf),
                "cxw": cx.astype(bf),
                "totw": tot.astype(bf),
                "biasw": bias,
            }
        )
    return in_maps


class _Runner:
    """Compile once, keep the jitted sharded executable for reuse."""

    def __init__(self):
        from jax.sharding import Mesh, PartitionSpec
        from jax.experimental.shard_map import shard_map

        self.nc = _build_module()
        nc = self.nc
        bass2jax.install_neuronx_cc_hook()

        part_name = (
            nc.partition_id_tensor.name if nc.partition_id_tensor else None
        )
        in_names, out_names, out_avals, zero_shapes = [], [], [], []
        for alloc in nc.m.functions[0].allocations:
            if not isinstance(alloc, mybir.MemoryLocationSet):
                continue
            name = alloc.memorylocations[0].name
            if alloc.kind == "ExternalInput":
                if name != part_name:
                    in_names.append(name)
            elif alloc.kind == "ExternalOutput":
                out_names.append(name)
                shape = tuple(alloc.tensor_shape)
                dtype = mybir.dt.np(alloc.dtype)
                out_avals.append(jax.core.ShapedArray(shape, dtype))
                zero_shapes.append((shape, dtype))
        n_params = len(in_names)
        n_outs = len(out_names)
        all_names = in_names + out_names
        if part_name is not None:
            all_names = all_names + [part_name]
        self.in_names = in_names
        self.out_names = out_names
        self.n_params = n_params
        self.zero_shapes = zero_shapes

        def _body(*args):
            operands = list(args)
            if part_name is not None:
                operands.append(bass2jax.partition_id_tensor())
            outs = bass2jax._bass_exec_p.bind(
                *operands,
                out_avals=tuple(out_avals),
                in_names=tuple(all_names),
                out_names=tuple(out_names),
                lowering_input_output_aliases=(),
                sim_require_finite=True,
                sim_require_nnan=True,
                nc=nc,
            )
            return tuple(outs)

        devices = jax.devices()[:N_CORES]
        mesh = Mesh(np.asarray(devices), ("core",))
        in_specs = (PartitionSpec("core"),) * (n_params + n_outs)
        out_specs = (PartitionSpec("core"),) * n_outs
        self.fn = jax.jit(
            shard_map(
                _body, mesh=mesh, in_specs=in_specs, out_specs=out_specs,
                check_rep=False,
            ),
            donate_argnums=tuple(range(n_params, n_params + n_outs)),
            keep_unused=True,
        )

    def __call__(self, in_maps):
        concat_in = [
            np.concatenate([np.asarray(m[name]) for m in in_maps], axis=0)
            for name in self.in_names
        ]
        zeros = [
            np.zeros((N_CORES * s[0], *s[1:]), d) for s, d in self.zero_shapes
        ]
        outs = self.fn(*concat_in, *zeros)
        return [
            {
                name: np.asarray(outs[i]).reshape(N_CORES, -1, *outs[i].shape[1:])[c]
                for i, name in enumerate(self.out_names)
            }
            for c in range(N_CORES)
        ]


def kernel(x, t, alpha_ratio, et_coeff, et_prevsum_coeff, conv_w, temb):
    global _compiled
    if _compiled is None:
        _compiled = _Runner()

    in_maps = _build_inputs(x, alpha_ratio, et_coeff, et_prevsum_coeff, conv_w, temb, t)
    results = _compiled(in_maps)

    x = np.asarray(x, np.float32)
    y = np.empty((T + 1, C, 64, 64), np.float32)
    y[0] = x[0]
    gs = np.arange(G)
    for k in range(N_CORES):
        o = k * TLOC
        oa = results[k]["out_arr"]  # [128, S, HW]
        for j in range(S):
            gv = gs[3 * gs + j <= TLOC - 1]
            if j == S - 1:
                # shifted layout: partition group g+1 holds image 3g+2
                gp = gv + 1
                rows = o + 3 * gp  # = o + (3g+2) + 1
                y[rows] = oa[(3 * gp[:, None] + np.arange(C)), j].reshape(
                    len(gp), C, 64, 64
                )
            else:
                rows = o + 3 * gv + j + 1
                y[rows] = oa[(3 * gv[:, None] + np.arange(C)), j].reshape(
                    len(gv), C, 64, 64
                )
    return y



# revision 2
# speedup vs baseline: 1.1582x; 1.1582x over previous
"""Trainium2 Bass kernel for nn_DEQLatentSpaceOpt (DDIM trajectory DEQ iteration).

Computation (reference):
    xT = x[0:1]
    repeat 3x:  et = conv3x3(xt[:T]) + temb[t][:,:,None,None]
                xt_next = alpha_ratio*xT + epc * cumsum(et_coeff * et, axis=0)
                xt = concat([xT, xt_next])

The whole computation is linear, so the three iterations are linearized
host-side into
    out[1:] = M @ C^3(x[0:T]) + N @ basis
with M = Dc K^2 (Dc = D_epc tril D_etc, K = shift(Dc)) a dense lower-
triangular TxT row-mixing matrix and N [T,12] host-precomputed coefficients
onto 12 basis images (xT, C xT, C^2 xT, conv responses of the per-channel
temb bias, and per-channel constants).  The device work is then:
  - 3 chained 3x3 convs over the core's 125 trajectory rows, computed in a
    transposed layout (partitions = (y,c), free = (img, x) with 1-px gaps):
    dy and channel mixing live in the stationary matrix (y-banded block
    weights), dx is 3 shifted-rhs accumulating matmuls -> 3 passes/conv
    instead of 9.
  - one PE transpose of z3 into image-major layout [125 imgs, (x,y,c)].
  - ONE combine: out = M_loc @ z3' (single 125-contraction matmul per
    512-col chunk) + cross/basis matmul.  Cross-core coupling of M is
    exactly rank 3 per boundary (SVD'd host-side); each core AllGathers 3
    weighted z3-sums (3 image rows), so one collective per run instead of 3.

Sharding: trajectory rows, 125 per core across 8 cores.
"""

import numpy as np
import ml_dtypes

import jax
import concourse.bacc as bacc
import concourse.mybir as mybir
import concourse.tile as tile
from concourse.bass_interp import get_hw_module
from concourse import bass2jax

BF16 = mybir.dt.bfloat16
F32 = mybir.dt.float32

N_CORES = 8
T = 1000
C = 3
TL = T // N_CORES  # 125 rows (images) per core
IMW = 65           # flat column stride per image (64 px + 1 shared pad)
NIMG = 126         # image slots in the flat layout (125 used + 1 pad slot)
NCOLS = NIMG * IMW  # 8190 flat columns per partition
FR = 192 * 64      # z3' / basis free size: (x, y, c) = 12288
NCH = FR // 512    # 24 combine chunks

# conv y-split: zA holds y 0..41 (partition p = 3y+c, 126), zB holds
# y 40..63 (p = 3(y-40)+c, 72).  Block1 outputs y 0..40 at psum rows
# 0..122; block2 outputs y 41..63 at psum rows 3..71 (rows 0..2 are zero
# weight columns) so both evacs are partition-aligned.  The two halo rows
# (y41 -> zA rows 123..125, y40 -> zB rows 0..2) are patched with
# SBUF-to-SBUF DMAs, batched per quarter-conv.
PA_IN, PA_OUT = 126, 123
PB_IN, PB_OUT = 72, 72

_compiled = None


def _windows():
    ws = []
    i0 = 0
    while i0 < TL:
        ni = min(7, TL - i0)
        ws.append((i0, ni))
        i0 += ni
    return ws


def _build_module(sim_mode=False):
    """sim_mode: single-core variant with the AllGather replaced by
    byte-equivalent local DMAs, for TimelineSim cost estimation only."""
    nc = bacc.Bacc(
        "TRN2",
        target_bir_lowering=False,
        debug=False,
        num_devices=1 if sim_mode else N_CORES,
    )

    # I/O
    xa = nc.dram_tensor("xa", [128, NCOLS], BF16, kind="ExternalInput").ap()
    xb = nc.dram_tensor("xb", [PB_IN, NCOLS], BF16, kind="ExternalInput").ap()
    cw1 = nc.dram_tensor("cw1", [PA_IN, 3, PA_OUT], BF16, kind="ExternalInput").ap()
    cw2 = nc.dram_tensor("cw2", [PB_IN, 3, PB_OUT], BF16, kind="ExternalInput").ap()
    mlocw = nc.dram_tensor("mlocw", [TL, TL], BF16, kind="ExternalInput").ap()
    totw = nc.dram_tensor("totw", [TL, 2, 6], BF16, kind="ExternalInput").ap()
    crossw = nc.dram_tensor("crossw", [72, 2, TL], BF16, kind="ExternalInput").ap()
    basisw = nc.dram_tensor("basisw", [24, FR // 2], BF16, kind="ExternalInput").ap()
    idw = nc.dram_tensor("idw", [128, 128], BF16, kind="ExternalInput").ap()
    out_arr = nc.dram_tensor("out_arr", [128, FR], BF16, kind="ExternalOutput").ap()

    WS = _windows()
    # quarter boundaries (in image index) for halo DMA batching
    QS = [(0, 35), (35, 63), (63, 98), (98, 125)]

    with tile.TileContext(nc) as tc:
        with (
            tc.tile_pool(name="persist", bufs=1) as pp,
            tc.tile_pool(name="pA", bufs=3, space="PSUM") as pA,
            tc.tile_pool(name="pB", bufs=3, space="PSUM") as pB,
            tc.tile_pool(name="pC", bufs=2, space="PSUM") as pC,
            tc.tile_pool(name="ring", bufs=3) as rp,
            tc.tile_pool(name="dram", bufs=2, space="DRAM") as dp,
        ):
            zA0 = pp.tile([128, NCOLS], BF16, tag="zA0")
            zA1 = pp.tile([128, NCOLS], BF16, tag="zA1")
            zB0 = pp.tile([PB_IN, NCOLS], BF16, tag="zB0")
            zB1 = pp.tile([PB_IN, NCOLS], BF16, tag="zB1")
            z3p = pp.tile([TL, FR], BF16, tag="z3p")
            bas = pp.tile([72, FR // 2], BF16, tag="bas")
            agin = pp.tile([6, FR // 2], BF16, tag="agin")
            cw1s = pp.tile([PA_IN, 3, PA_OUT], BF16, tag="cw1s")
            cw2s = pp.tile([PB_IN, 3, PB_OUT], BF16, tag="cw2s")
            mlocs = pp.tile([TL, TL], BF16, tag="mlocs")
            tots = pp.tile([TL, 2, 6], BF16, tag="tots")
            crs = pp.tile([72, 2, TL], BF16, tag="crs")
            ids = pp.tile([128, 128], BF16, tag="ids")

            # conv weights + first input chunk first so conv1 starts ASAP;
            # xb goes through the Pool-engine SWDGE path so its descriptor
            # generation runs parallel to HWDGE; combine-phase weights are
            # loaded later (during conv1)
            nc.sync.dma_start(cw1s[:], cw1[:])
            nc.sync.dma_start(zA0[:, 0 : 14 * IMW], xa[:, 0 : 14 * IMW])
            nc.sync.dma_start(cw2s[:], cw2[:])
            for a, b in ((0, 14), (14, 42), (42, 77), (77, 126)):
                nc.gpsimd.dma_start(
                    zB0[:, a * IMW : b * IMW], xb[:, a * IMW : b * IMW]
                )
            for a, b in ((14, 42), (42, 77), (77, 126)):
                nc.sync.dma_start(
                    zA0[:, a * IMW : b * IMW], xa[:, a * IMW : b * IMW]
                )

            # zero the pad columns of the pong buffers once (conv evacs never
            # write them; ping pads come zeroed from the host)
            for z, np_ in ((zA1, 128), (zB1, PB_IN)):
                zr = z.rearrange("p (i w) -> p i w", w=IMW)
                nc.gpsimd.memset(zr[0:np_, :, 0:1], 0.0)
                nc.gpsimd.memset(zr[0:np_, TL : TL + 1, 1:], 0.0)

            # ---- 3 chained convs ----
            for cv in range(3):
                if cv % 2 == 0:
                    sa, sb, da, db = zA0, zB0, zA1, zB1
                else:
                    sa, sb, da, db = zA1, zB1, zA0, zB0
                dar = da.rearrange("p (i w) -> p i w", w=IMW)
                dbr = db.rearrange("p (i w) -> p i w", w=IMW)
                qi = 0
                for wi, (i0, ni) in enumerate(WS):
                    s = i0 * IMW + 1
                    wd = ni * IMW - 1
                    p1 = pA.tile([PA_OUT, 455], F32, tag="p1")
                    p2 = pB.tile([PB_OUT, 455], F32, tag="p2")
                    for di, d in enumerate((-1, 0, 1)):
                        nc.tensor.matmul(
                            p1[:, :wd],
                            cw1s[:, di],
                            sa[0:PA_IN, s + d : s + d + wd],
                            start=(di == 0),
                            stop=(di == 2),
                        )
                    for di, d in enumerate((-1, 0, 1)):
                        nc.tensor.matmul(
                            p2[:, :wd],
                            cw2s[:, di],
                            sb[0:PB_IN, s + d : s + d + wd],
                            start=(di == 0),
                            stop=(di == 2),
                        )
                    p1v = p1.rearrange("p (i w) -> p i w", w=IMW)[:, :ni, :64]
                    p2v = p2.rearrange("p (i w) -> p i w", w=IMW)[:, :ni, :64]
                    # y 0..40 -> zA rows 0..122; y 41..63 -> zB rows 3..71
                    # (rows 0..2 of p2 are zeros); alternate ACT/DVE
                    # p2 rows 0..2 are computed zeros; the halo DMA below
                    # overwrites dbr rows 0..2 with the real y40 afterwards.
                    # Fixed engine assignment keeps each psum slot's WAR
                    # chain on one engine.
                    nc.scalar.activation(
                        dar[0:PA_OUT, i0 : i0 + ni, 1:65],
                        p1v,
                        mybir.ActivationFunctionType.Copy,
                    )
                    nc.vector.tensor_copy(
                        dbr[0:PB_OUT, i0 : i0 + ni, 1:65], p2v
                    )
                    # halo rows via SBUF->SBUF DMA, batched per quarter
                    # (skipped for conv3: the transpose reads avoid them)
                    if cv < 2 and qi < 4 and i0 + ni >= QS[qi][1]:
                        a, b = QS[qi][0] * IMW, QS[qi][1] * IMW
                        nc.sync.dma_start(db[0:3, a:b], da[120:123, a:b])
                        nc.sync.dma_start(da[123:126, a:b], db[3:6, a:b])
                        qi += 1
                if cv == 0:
                    # combine-phase weights, loaded while conv2/3 run
                    nc.sync.dma_start(ids[:], idw[:])
                    nc.sync.dma_start(mlocs[:], mlocw[:])
                    nc.sync.dma_start(tots[:], totw[:])
                    nc.sync.dma_start(crs[:], crossw[:])
                    nc.sync.dma_start(bas[48:72], basisw[:])

            # ---- transpose z3 (in zA1/zB1) -> z3p [img, (x, y, c)] ----
            # y 0..40 from zA rows 0..122, y 41..63 from zB rows 3..71 --
            # neither touches the (unwritten) conv3 halo rows
            zar = zA1.rearrange("p (i w) -> p i w", w=IMW)
            zbr = zB1.rearrange("p (i w) -> p i w", w=IMW)
            for g in range(16):
                ptr = pC.tile([TL, 4 * 198], BF16, tag="pc")
                for xi in range(4):
                    x = 4 * g + xi
                    nc.tensor.matmul(
                        ptr[:, xi * 198 : xi * 198 + 123],
                        zar[0:123, 0:TL, 1 + x : 2 + x],
                        ids[0:123, 0:123],
                        is_transpose=True,
                    )
                    nc.tensor.matmul(
                        ptr[:, xi * 198 + 126 : xi * 198 + 198],
                        zbr[0:PB_IN, 0:TL, 1 + x : 2 + x],
                        ids[0:PB_IN, 0:PB_IN],
                        is_transpose=True,
                    )
                pv = ptr.rearrange("p (a b) -> p a b", b=198)
                zv = z3p[:, g * 768 : (g + 1) * 768].rearrange(
                    "p (a b) -> p a b", b=192
                )
                nc.vector.tensor_copy(zv[:, :, 0:123], pv[:, :, 0:123])
                nc.scalar.activation(
                    zv[:, :, 123:192],
                    pv[:, :, 129:198],
                    mybir.ActivationFunctionType.Copy,
                )

            # ---- totals (3 weighted z3 sums) -> AllGather ----
            # chunk pairs (c = 2g+j) stack onto psum partitions (3j+f): each
            # psum tile covers 2 chunks and evacs once; the j-selection is
            # folded into the totals/cross stationary weights, so the
            # gathered payload needs no permutation.
            pools = (pA, pB, pC)
            tags = ("p1", "p2", "pc")
            for g in range(NCH // 2):
                pt = pools[g % 3].tile([6, 512], F32, tag=tags[g % 3])
                for j in range(2):
                    ch = 2 * g + j
                    sl = slice(ch * 512, (ch + 1) * 512)
                    nc.tensor.matmul(
                        pt[:], tots[:, j], z3p[:, sl],
                        start=(j == 0), stop=(j == 1),
                    )
                if g % 2 == 0:
                    nc.scalar.activation(
                        agin[:, g * 512 : (g + 1) * 512],
                        pt[:],
                        mybir.ActivationFunctionType.Copy,
                    )
                else:
                    nc.vector.tensor_copy(
                        agin[:, g * 512 : (g + 1) * 512], pt[:]
                    )
            ag_in = dp.tile([6, FR // 2], BF16, tag="ag_in")
            ag_out = dp.tile([N_CORES * 6, FR // 2], BF16, tag="ag_out")
            nc.sync.dma_start(ag_in[:], agin[:])
            if sim_mode:
                # byte-equivalent local substitute: one repeat-read DMA
                nc.sync.dma_start(
                    ag_out.rearrange("(a b) f -> a b f", b=6),
                    ag_in.rearrange("(x a) f -> x a f", x=1).broadcast_to(
                        [N_CORES, 6, FR // 2]
                    ),
                )
            else:
                nc.gpsimd.collective_compute(
                    "AllGather",
                    mybir.AluOpType.bypass,
                    replica_groups=[list(range(N_CORES))],
                    ins=[ag_in.opt()],
                    outs=[ag_out.opt()],
                )
            nc.sync.dma_start(bas[0:48], ag_out[:])

            # ---- combine: local triangular mix + cross/basis, stream out ----
            # local matmuls of chunks 0..5 are issued first so the PE has
            # work while the AllGather chain completes; their cross matmuls
            # + evacs follow, then the rest runs single-pass.
            PRE = 8
            pcs = []
            for ch in range(PRE):
                sl = slice(ch * 512, (ch + 1) * 512)
                pc = pools[ch % 3].tile([TL, 512], F32, tag=tags[ch % 3])
                nc.tensor.matmul(
                    pc[:], mlocs[:], z3p[:, sl], start=True, stop=False
                )
                pcs.append(pc)
            rgs = {}
            for ch in range(NCH):
                rc, ci = divmod(ch, 2)
                if ci == 0:
                    rgs[rc] = rp.tile([TL, 1024], BF16, tag="ring", name="rg")
                rg = rgs[rc]
                sl = slice(ch * 512, (ch + 1) * 512)
                if ch < PRE:
                    pc = pcs[ch]
                else:
                    pc = pools[ch % 3].tile([TL, 512], F32, tag=tags[ch % 3])
                    nc.tensor.matmul(
                        pc[:], mlocs[:], z3p[:, sl], start=True, stop=False
                    )
                gp, jp = divmod(ch, 2)
                nc.tensor.matmul(
                    pc[:],
                    crs[:, jp],
                    bas[:, gp * 512 : (gp + 1) * 512],
                    start=False,
                    stop=True,
                )
                if ch % 2 == 0:
                    nc.scalar.activation(
                        rg[:, ci * 512 : (ci + 1) * 512],
                        pc[:],
                        mybir.ActivationFunctionType.Copy,
                    )
                else:
                    nc.vector.tensor_copy(
                        rg[:, ci * 512 : (ci + 1) * 512], pc[:]
                    )
                if ci == 1:
                    nc.sync.dma_start(
                        out_arr[0:TL, rc * 1024 : (rc + 1) * 1024], rg[:]
                    )

    nc.compile()
    nc.m = get_hw_module(nc.m)
    return nc


def _build_inputs(x, alpha_ratio, et_coeff, et_prevsum_coeff, conv_w, temb, t):
    """Host-side linearization; returns per-core in_maps."""
    ar = np.asarray(alpha_ratio, np.float64).reshape(T)
    etc = np.asarray(et_coeff, np.float64).reshape(T)
    epc = np.asarray(et_prevsum_coeff, np.float64).reshape(T)
    temb = np.asarray(temb, np.float64)
    t = np.asarray(t).astype(np.int64)
    conv_w = np.asarray(conv_w, np.float64)
    x = np.asarray(x, np.float64)
    temb_sel = temb[t]
    bf = ml_dtypes.bfloat16

    def conv3(img):  # [N, C, 64, 64] -> same, SAME zero pad
        out = np.zeros_like(img)
        pad = np.pad(img, ((0, 0), (0, 0), (1, 1), (1, 1)))
        for co in range(C):
            for ci in range(C):
                for dy in range(3):
                    for dx in range(3):
                        out[:, co] += (
                            conv_w[co, ci, dy, dx]
                            * pad[:, ci, dy : dy + 64, dx : dx + 64]
                        )
        return out

    # row-mixing matrices (fp64)
    ii = np.arange(T)
    Dc = (epc[:, None] * etc[None, :]) * (ii[None, :] <= ii[:, None])
    K = np.zeros((T, T))
    K[1:] = Dc[:-1]
    DcK = Dc @ K
    M = DcK @ K

    a_vec = ar
    ahat = np.zeros(T)
    ahat[0] = 1.0
    ahat += np.concatenate([[0.0], a_vec[:-1]])

    xT = x[0]
    CxT = conv3(xT[None])[0]
    C2xT = conv3(CxT[None])[0]
    ones_c = np.zeros((C, C, 64, 64))
    for c in range(C):
        ones_c[c, c] = 1.0
    E = conv3(ones_c)
    E2 = conv3(E)

    Ncols = [a_vec, Dc @ ahat, DcK @ ahat]
    Ncols += [DcK @ temb_sel[:, c] for c in range(C)]
    Ncols += [M @ temb_sel[:, c] for c in range(C)]
    Ncols += [Dc @ temb_sel[:, c] for c in range(C)]
    Ncoef = np.stack(Ncols, 1)  # [T, 12] order: xT,CxT,C2xT,E*3,E2*3,ones*3

    basis = np.stack(
        [xT, CxT, C2xT, E[0], E[1], E[2], E2[0], E2[1], E2[2]]
        + [ones_c[c] for c in range(C)],
        0,
    )  # [12, C, 64, 64]
    # basis in the chunk-paired layout: row (2b+j), col (g*512+x) holds
    # basis[b, (2g+j)*512 + x]
    basis_flat = basis.transpose(0, 3, 2, 1).reshape(12, FR)
    basisw = np.ascontiguousarray(
        basis_flat.reshape(12, FR // 1024, 2, 512)
        .transpose(0, 2, 1, 3)
        .reshape(24, FR // 2)
    ).astype(bf)

    # cross-core rank-3 factors
    send_w = []
    recv_w = []
    for k in range(N_CORES):
        beta = k * TL
        if k < N_CORES - 1:
            blk = M[beta + TL :, beta : beta + TL]
            U, S, Vt = np.linalg.svd(blk, full_matrices=False)
            r = 3
            sq = np.sqrt(S[:r])
            send_w.append(Vt[:r].T * sq)        # [TL, 3]
            recv_w.append(U[:, :r] * sq)        # [T - beta - TL, 3]
        else:
            send_w.append(np.zeros((TL, 3)))
            recv_w.append(np.zeros((0, 3)))

    # shared conv weights, y-banded
    cw1 = np.zeros((3, PA_IN, PA_OUT))
    cw2 = np.zeros((3, PB_IN, PB_OUT))
    for dx in range(3):
        for yo in range(41):       # block1 out y 0..40 -> psum rows 0..122
            for ky in range(3):
                yi = yo + ky - 1
                if 0 <= yi <= 41:
                    for co in range(C):
                        for ci in range(C):
                            cw1[dx, 3 * yi + ci, 3 * yo + co] = conv_w[
                                co, ci, ky, dx
                            ]
        for yo in range(41, 64):   # block2 out y 41..63 -> psum rows 3..71
            for ky in range(3):
                yi = yo + ky - 1
                if 40 <= yi <= 63:
                    for co in range(C):
                        for ci in range(C):
                            cw2[
                                dx, 3 * (yi - 40) + ci, 3 * (yo - 40) + co
                            ] = conv_w[co, ci, ky, dx]
    cw1 = np.ascontiguousarray(cw1.transpose(1, 0, 2)).astype(bf)  # [126,3,123]
    cw2 = np.ascontiguousarray(cw2.transpose(1, 0, 2)).astype(bf)  # [72,3,72]

    idw = np.eye(128, dtype=np.float32).astype(bf)

    in_maps = []
    for k in range(N_CORES):
        o = k * TL
        xseg = x[o : o + TL].astype(np.float32)  # [125, C, 64, 64]

        xa = np.zeros((128, NIMG, IMW), np.float32)
        xa[0:PA_IN, :TL, 1:65] = (
            xseg[:, :, 0:42].transpose(2, 1, 0, 3).reshape(PA_IN, TL, 64)
        )
        xb = np.zeros((PB_IN, NIMG, IMW), np.float32)
        xb[:, :TL, 1:65] = (
            xseg[:, :, 40:64].transpose(2, 1, 0, 3).reshape(PB_IN, TL, 64)
        )

        mloc = M[o : o + TL, o : o + TL].T  # lhsT [j, i]

        # totals stationary: 2 variants, variant j writes psum rows 3j..3j+3
        totw_k = np.zeros((TL, 2, 6))
        for j in range(2):
            totw_k[:, j, 3 * j : 3 * j + 3] = send_w[k]

        # cross stationary [72, 2, TL]: variant j selects the j-parity rows
        # of the gathered payload (rows 6k'+3j+f) and basis (rows 48+2b+j)
        cross = np.zeros((72, 2, TL))
        for j in range(2):
            for kp in range(k):
                rw = recv_w[kp]
                off = o - (kp + 1) * TL
                cross[6 * kp + 3 * j : 6 * kp + 3 * j + 3, j, :] = rw[
                    off : off + TL
                ].T
            for b in range(12):
                cross[48 + 2 * b + j, j, :] = Ncoef[o : o + TL, b]

        in_maps.append(
            {
                "xa": xa.reshape(128, NCOLS).astype(bf),
                "xb": xb.reshape(PB_IN, NCOLS).astype(bf),
                "cw1": cw1,
                "cw2": cw2,
                "mlocw": mloc.astype(bf),
                "totw": totw_k.astype(bf),
                "crossw": cross.astype(bf),
                "basisw": basisw,
                "idw": idw,
            }
        )
    return in_maps


class _Runner:
    """Compile once, keep the jitted sharded executable for reuse."""

    def __init__(self):
        from jax.sharding import Mesh, PartitionSpec
        from jax.experimental.shard_map import shard_map

        self.nc = _build_module()
        nc = self.nc
        bass2jax.install_neuronx_cc_hook()

        part_name = (
            nc.partition_id_tensor.name if nc.partition_id_tensor else None
        )
        in_names, out_names, out_avals, zero_shapes = [], [], [], []
        for alloc in nc.m.functions[0].allocations:
            if not isinstance(alloc, mybir.MemoryLocationSet):
                continue
            name = alloc.memorylocations[0].name
            if alloc.kind == "ExternalInput":
                if name != part_name:
                    in_names.append(name)
            elif alloc.kind == "ExternalOutput":
                out_names.append(name)
                shape = tuple(alloc.tensor_shape)
                dtype = mybir.dt.np(alloc.dtype)
                out_avals.append(jax.core.ShapedArray(shape, dtype))
                zero_shapes.append((shape, dtype))
        n_params = len(in_names)
        n_outs = len(out_names)
        all_names = in_names + out_names
        if part_name is not None:
            all_names = all_names + [part_name]
        self.in_names = in_names
        self.out_names = out_names
        self.n_params = n_params
        self.zero_shapes = zero_shapes

        def _body(*args):
            operands = list(args)
            if part_name is not None:
                operands.append(bass2jax.partition_id_tensor())
            outs = bass2jax._bass_exec_p.bind(
                *operands,
                out_avals=tuple(out_avals),
                in_names=tuple(all_names),
                out_names=tuple(out_names),
                lowering_input_output_aliases=(),
                sim_require_finite=True,
                sim_require_nnan=True,
                nc=nc,
            )
            return tuple(outs)

        devices = jax.devices()[:N_CORES]
        mesh = Mesh(np.asarray(devices), ("core",))
        in_specs = (PartitionSpec("core"),) * (n_params + n_outs)
        out_specs = (PartitionSpec("core"),) * n_outs
        self.fn = jax.jit(
            shard_map(
                _body, mesh=mesh, in_specs=in_specs, out_specs=out_specs,
                check_rep=False,
            ),
            donate_argnums=tuple(range(n_params, n_params + n_outs)),
            keep_unused=True,
        )

    def __call__(self, in_maps):
        concat_in = [
            np.concatenate([np.asarray(m[name]) for m in in_maps], axis=0)
            for name in self.in_names
        ]
        zeros = [
            np.zeros((N_CORES * s[0], *s[1:]), d) for s, d in self.zero_shapes
        ]
        outs = self.fn(*concat_in, *zeros)
        return [
            {
                name: np.asarray(outs[i]).reshape(N_CORES, -1, *outs[i].shape[1:])[c]
                for i, name in enumerate(self.out_names)
            }
            for c in range(N_CORES)
        ]


def kernel(x, t, alpha_ratio, et_coeff, et_prevsum_coeff, conv_w, temb):
    global _compiled
    if _compiled is None:
        _compiled = _Runner()

    in_maps = _build_inputs(
        x, alpha_ratio, et_coeff, et_prevsum_coeff, conv_w, temb, t
    )
    results = _compiled(in_maps)

    x = np.asarray(x, np.float32)
    y = np.empty((T + 1, C, 64, 64), np.float32)
    y[0] = x[0]
    for k in range(N_CORES):
        o = k * TL
        oa = results[k]["out_arr"][0:TL].astype(np.float32)
        # [125, FR], f = x*192 + 3y + c
        y[1 + o : 1 + o + TL] = (
            oa.reshape(TL, 64, 64, C).transpose(0, 3, 2, 1)
        )
    return y
